# revision 14
# baseline (speedup 1.0000x reference)
"""Causal MHA with RoPE on 8 Trainium2 NeuronCores.

Sharding: core c -> batch b=c//2, head-group g=c%2 (8 heads of 16).
Each core: Q/K/V projections for its 512 head-dims over the full sequence,
causal attention for its 8 heads, partial output projection (its 512 rows
of wo). Host sums the two partial outputs per batch. No collectives.

All matmuls run in float32r (full-rate PE mode, ~1.5e-4 rel err at K=1024).
RoPE is applied via host-permuted wq/wk columns ([evens|odds] per head),
32-row block-swap DMAs and precomputed cos/sin tables.
Softmax skips max-subtraction (scores are O(1) after the 1/8 scale), uses
an additive -1e9 causal mask on diagonal tiles, and gets denominators from
a ones-column appended to V (M=65 AV matmul).
"""

import math

import numpy as np

import concourse.bass as bass
import concourse.mybir as mybir
import concourse.tile as tile
from concourse import bacc
from concourse.bass_utils import run_bass_kernel_spmd
from concourse.masks import make_identity

F32 = mybir.dt.float32
F32R = mybir.dt.float32r

B, S, D, H = 4, 2048, 1024, 16
HD = D // H          # 64
THETA = 10000.0
DH = D // 2          # 512 per-core head dims (8 heads)
NP = 4               # head pairs per core
NTH = 4              # token passes for x^T materialization / projections
THT = S // NTH       # 512 tokens per pass
NQB = 4              # query blocks of 512
QB = S // NQB
NKT = S // 128       # 16 key tiles of 128
SCALE = 1.0 / math.sqrt(HD)
NEG = -1.0e9

_cached = None


def _build():
    nc = bacc.Bacc(None, target_bir_lowering=False)

    x = nc.dram_tensor("x", [S, D], F32, kind="ExternalInput")
    wk = nc.dram_tensor("wk", [D, DH], F32, kind="ExternalInput")
    wq = nc.dram_tensor("wq", [D, DH], F32, kind="ExternalInput")
    wv = nc.dram_tensor("wv", [D, DH], F32, kind="ExternalInput")
    wo = nc.dram_tensor("wo", [DH, D], F32, kind="ExternalInput")
    cosb = nc.dram_tensor("cosb", [128, S], F32, kind="ExternalInput")
    sinb = nc.dram_tensor("sinb", [128, S], F32, kind="ExternalInput")
    outp = nc.dram_tensor("outp", [S, D], F32, kind="ExternalOutput")

    with tile.TileContext(nc) as tc:
        with (
            tc.tile_pool(name="const", bufs=1) as cpool,
            tc.tile_pool(name="kq", bufs=1) as kqpool,
            tc.tile_pool(name="vaug", bufs=1) as vpool,
            tc.tile_pool(name="xt", bufs=8) as xtpool,
            tc.tile_pool(name="stream", bufs=2) as spool,
            tc.tile_pool(name="w512", bufs=8) as wpool,
        ):
            ident = cpool.tile([128, 128], F32, name="ident")
            make_identity(nc, ident)
            tri = cpool.tile([128, 128], F32, name="tri")
            nc.gpsimd.memset(tri, 0.0)
            # tri[i, t] = 0 if t >= i else NEG  (mask k>q inside diagonal tiles)
            nc.gpsimd.affine_select(
                out=tri, in_=tri, compare_op=mybir.AluOpType.is_ge,
                fill=NEG, base=0, pattern=[[1, 128]], channel_multiplier=-1,
            )
            cos_t = cpool.tile([128, S], F32, name="cos_t")
            sin_t = cpool.tile([128, S], F32, name="sin_t")
            nc.sync.dma_start(out=cos_t, in_=cosb[:, :])
            nc.sync.dma_start(out=sin_t, in_=sinb[:, :])

            # K^T / Q^T pair tiles: [128 dims (head 2p | head 2p+1), S tokens]
            kt_tiles = [kqpool.tile([128, S], F32R, name=f"ktp{p}", tag=f"ktp{p}") for p in range(NP)]
            qt_tiles = [kqpool.tile([128, S], F32R, name=f"qtp{p}", tag="qc", bufs=8) for p in range(NP)]
            ctx_tiles = [kqpool.tile([128, S], F32R, name=f"ctxp{p}", tag="qc", bufs=8) for p in range(NP)]
            # V tiles with ones column: [128 tokens, 8 heads, 64+1]
            v_tiles = [vpool.tile([128, 8, HD + 1], F32R, name=f"vt{t}", tag=f"vt{t}") for t in range(NKT)]
            for t in range(NKT):
                # ones column via exp(0*x) = 1
                nc.scalar.activation(
                    v_tiles[t][:, :, HD], cos_t[:, 0:8],
                    mybir.ActivationFunctionType.Exp, scale=0.0,
                )

            with (
                tc.tile_pool(name="pst", bufs=2, space="PSUM") as pst,
                tc.tile_pool(name="pssc", bufs=2, space="PSUM") as pssc,
                tc.tile_pool(name="psc", bufs=1, space="PSUM") as psc,
            ):
                for th in range(NTH):
                    t0 = th * THT
                    # ---- x^T materialization for this token pass ----
                    xtb = [xtpool.tile([128, 4, THT], F32R, name=f"xtb{th}_{h}", tag="xt", bufs=2)
                           for h in range(2)]
                    xts = [xtb[dc // 4][:, dc % 4, :] for dc in range(8)]
                    for tl in range(THT // 128):
                        for hf in range(2):
                            xl = spool.tile([128, D // 2], F32, name="xl", tag="xl")
                            nc.sync.dma_start(
                                out=xl,
                                in_=x[t0 + tl * 128 : t0 + (tl + 1) * 128,
                                      hf * 512 : (hf + 1) * 512])
                            tp = pst.tile([128, 512], F32, name="tp", tag="tp")
                            for dq in range(4):
                                nc.tensor.matmul(
                                    tp[:, dq * 128 : (dq + 1) * 128],
                                    xl[:, dq * 128 : (dq + 1) * 128], ident,
                                    is_transpose=True,
                                    start=(dq == 0), stop=(dq == 3))
                            nc.vector.tensor_copy(
                                xtb[hf][:, :, tl * 128 : (tl + 1) * 128],
                                tp.rearrange("a (c d) -> a c d", c=4))

                    # ---- K^T / Q^T projections + RoPE for this token pass ----
                    for wmat, dst in ((wk, kt_tiles), (wq, qt_tiles)):
                        wb = wpool.tile([128, 8, DH], F32R, name="wb", tag="wbig", bufs=1)
                        nc.gpsimd.dma_start(out=wb, in_=wmat.rearrange("(c p) j -> p c j", p=128))
                        for p in range(NP):
                            acc = pst.tile([128, THT], F32, name="acc", tag="tp")
                            for dc in range(8):
                                nc.tensor.matmul(
                                    acc, wb[:, dc, p * 128 : (p + 1) * 128], xts[dc],
                                    start=(dc == 0), stop=(dc == 7),
                                )
                            # rope fused with psum evacuation:
                            #   dst = acc*C - swap(acc*S)   (C swap-symmetric, S anti-symmetric)
                            dslice = dst[p][:, t0 : t0 + THT]
                            nc.vector.tensor_mul(dslice, acc, cos_t[:, t0 : t0 + THT])
                            raw = spool.tile([128, THT], F32, name="raw", tag="raw", bufs=2)
                            nc.vector.tensor_mul(raw, acc, sin_t[:, t0 : t0 + THT])
                            swp = spool.tile([128, THT], F32, name="swp", tag="swp", bufs=1)
                            nc.sync.dma_start(out=swp[0:32, :], in_=raw[32:64, :])
                            nc.sync.dma_start(out=swp[32:64, :], in_=raw[0:32, :])
                            nc.sync.dma_start(out=swp[64:96, :], in_=raw[96:128, :])
                            nc.sync.dma_start(out=swp[96:128, :], in_=raw[64:96, :])
                            nc.vector.tensor_sub(dslice, dslice, swp)

                    # ---- V projection for this token pass ----
                    wvb = wpool.tile([128, 8, DH], F32R, name="wvb", tag="wbig", bufs=1)
                    nc.gpsimd.dma_start(out=wvb, in_=wv.rearrange("(c p) j -> p c j", p=128))
                    for tl in range(THT // 128):
                        acc = pst.tile([128, DH], F32, name="vacc", tag="tp")
                        for dc in range(8):
                            nc.tensor.matmul(
                                acc, xts[dc][:, tl * 128 : (tl + 1) * 128], wvb[:, dc, :],
                                start=(dc == 0), stop=(dc == 7),
                            )
                        vt = v_tiles[th * (THT // 128) + tl]
                        # strided write: psum [128, 8*64] -> v_aug[:, h, 0:64]
                        nc.vector.tensor_copy(
                            vt[:, :, 0:HD],
                            acc.rearrange("a (h d) -> a h d", h=8),
                        )

                    # ---- attention for query block qb == th (all pairs) ----
                    qb = th
                    for p in range(NP):
                        ktp, qtp = kt_tiles[p], qt_tiles[p]
                        q0 = qb * QB
                        nk = 4 * qb + 4
                        pse = psc.tile([HD + 1, QB], F32, name="pse", tag="ctxe")
                        pso = psc.tile([HD + 1, QB], F32, name="pso", tag="ctxo")
                        for kt in range(nk):
                            dj = kt - (nk - 4)
                            qoff = 128 * dj if dj > 0 else 0
                            n = QB - qoff
                            psab = pssc.tile([128, 2 * QB], F32, name="psab", tag="sc")
                            ksl = slice(kt * 128, (kt + 1) * 128)
                            qsl = slice(q0 + qoff, q0 + QB)
                            nc.tensor.matmul(psab[:, 0:n], ktp[0:64, ksl], qtp[0:64, qsl])
                            nc.tensor.matmul(psab[:, QB : QB + n], ktp[64:128, ksl], qtp[64:128, qsl])
                            if dj >= 0:
                                mview = bass.AP(
                                    tensor=psab.tensor, offset=psab.offset,
                                    ap=[psab.ap[0], [QB, 2], [1, 128]])
                                tview = bass.AP(
                                    tensor=tri.tensor, offset=tri.offset,
                                    ap=[tri.ap[0], [0, 2], [1, 128]])
                                nc.vector.tensor_add(mview, mview, tview)
                            eab = spool.tile([128, 2 * QB], F32R, name="eab", tag="eab")
                            eview_o = bass.AP(
                                tensor=eab.tensor, offset=eab.offset,
                                ap=[eab.ap[0], [QB, 2], [1, n]])
                            eview_i = bass.AP(
                                tensor=psab.tensor, offset=psab.offset,
                                ap=[psab.ap[0], [QB, 2], [1, n]])
                            nc.scalar.activation(
                                eview_o, eview_i, mybir.ActivationFunctionType.Exp, scale=SCALE)
                            vt = v_tiles[kt]
                            nc.tensor.matmul(
                                pse[:, qoff:QB], vt[:, 2 * p, :], eab[:, 0:n],
                                start=(kt == 0), stop=(kt == nk - 1))
                            nc.tensor.matmul(
                                pso[:, qoff:QB], vt[:, 2 * p + 1, :], eab[:, QB : QB + n],
                                start=(kt == 0), stop=(kt == nk - 1))
                        for par, psx in ((0, pse), (1, pso)):
                            s0 = spool.tile([1, QB], F32, name="s0", tag="s0", bufs=1)
                            nc.vector.tensor_copy(s0[0:1, :], psx[HD : HD + 1, :])
                            nc.vector.reciprocal(s0[0:1, :], s0[0:1, :])
                            bc = spool.tile([HD, QB], F32, name="bc", tag="bc", bufs=2)
                            nc.gpsimd.partition_broadcast(bc, s0[0:1, :])
                            nc.vector.tensor_mul(
                                ctx_tiles[p][par * HD : (par + 1) * HD, q0 : q0 + QB],
                                psx[0:HD, :], bc)

            # ---------------- output projection ----------------
            with tc.tile_pool(name="pso2", bufs=4, space="PSUM") as pso2p:
                wo_r = wo.rearrange("(c p) j -> p c j", p=128)
                for nn in range(2):
                    wob = wpool.tile([128, 4, 512], F32R, name="wob", tag="wbig", bufs=1)
                    nc.gpsimd.dma_start(out=wob, in_=wo_r[:, :, nn * 512 : (nn + 1) * 512])
                    for t in range(NKT):
                        acc = pso2p.tile([128, 512], F32, name="oacc", tag="oacc")
                        for pc in range(4):
                            nc.tensor.matmul(
                                acc, ctx_tiles[pc][:, t * 128 : (t + 1) * 128],
                                wob[:, pc, :], start=(pc == 0), stop=(pc == 3))
                        osb = spool.tile([128, 512], F32, name="osb", tag="osb", bufs=2)
                        nc.scalar.copy(osb, acc)
                        nc.sync.dma_start(
                            out=outp[t * 128 : (t + 1) * 128, nn * 512 : (nn + 1) * 512], in_=osb)

    nc.compile()
    return nc


def _host_tables(token_positions):
    pos = np.asarray(token_positions, dtype=np.float64)
    inv_freq = np.exp(np.arange(0, HD, 2, dtype=np.float64) * (-math.log(THETA) / HD))  # [32]
    ang = pos[:, None] * inv_freq[None, :]  # [S, 32]
    cos = np.cos(ang).astype(np.float32).T  # [32, S]
    sin = np.sin(ang).astype(np.float32).T
    # pair-tile row layout: [head_even: 32 evens | 32 odds][head_odd: same]
    C = np.empty((128, S), np.float32)
    Sx = np.empty((128, S), np.float32)
    for half in range(2):
        r0 = 64 * half
        C[r0 : r0 + 32] = cos
        C[r0 + 32 : r0 + 64] = cos
        Sx[r0 : r0 + 32] = -sin
        Sx[r0 + 32 : r0 + 64] = sin
    return C, Sx


def kernel(in_features, token_positions, wq, wk, wv, wo):
    global _cached
    if _cached is None:
        _cached = _build()
    nc = _cached

    x = np.ascontiguousarray(in_features, dtype=np.float32)
    # permute wq/wk columns within each head: [evens | odds]
    perm = np.concatenate(
        [64 * h + np.concatenate([np.arange(0, 64, 2), np.arange(1, 64, 2)]) for h in range(H)])
    wqp = np.ascontiguousarray(wq[:, perm], dtype=np.float32)
    wkp = np.ascontiguousarray(wk[:, perm], dtype=np.float32)
    wv = np.ascontiguousarray(wv, dtype=np.float32)
    wo = np.ascontiguousarray(wo, dtype=np.float32)
    C, Sx = _host_tables(token_positions)

    in_maps = []
    for c in range(8):
        b, g = c // 2, c % 2
        sl = slice(g * DH, (g + 1) * DH)
        in_maps.append({
            "x": np.ascontiguousarray(x[b]),
            "wq": np.ascontiguousarray(wqp[:, sl]),
            "wk": np.ascontiguousarray(wkp[:, sl]),
            "wv": np.ascontiguousarray(wv[:, sl]),
            "wo": np.ascontiguousarray(wo[sl, :]),
            "cosb": C,
            "sinb": Sx,
        })
    res = run_bass_kernel_spmd(nc, in_maps, core_ids=list(range(8)))
    out = np.empty((B, S, D), np.float32)
    for b in range(B):
        out[b] = res.results[2 * b]["outp"] + res.results[2 * b + 1]["outp"]
    return out
